# revision 1
# baseline (speedup 1.0000x reference)
"""DGI (2-layer GCN encoder + bilinear discriminator) on 8 TRN2 NeuronCores.

Sharding: nodes (and edges by destination) across 8 cores. Per layer, each
core computes its shard of h @ W (pos|neg fused on the feature axis), the
full feature table is AllGathered in 4 bucket slices (each <=32K rows so
int16 gather indices work, and so aggregation overlaps the collectives),
then the segment-sum aggregation for the core's destination shard runs as
dma_gather row gathers + one-DVE-op selection matrices + PSUM-accumulated
matmuls. Gathers are truncated per-core via -1 index padding and a
num_idxs register loaded from an input, so padding costs no HBM traffic.
The graph summary is a masked column-sum + AllReduce; scores come from a
PE transpose + matvec against Wb @ sigmoid(mean H).
"""
import sys
sys.path.insert(0, "/opt/trn_rl_repo")

import numpy as np
import concourse.bass as bass
import concourse.bacc as bacc
import concourse.tile as tile
from concourse import bass_utils, mybir
from concourse.masks import make_identity
from concourse.tile import add_dep_helper

F32 = mybir.dt.float32
BF16 = mybir.dt.bfloat16
I16 = mybir.dt.int16
I32 = mybir.dt.int32

GATHER_BF16 = True   # feature tables gathered by edges + collectives in bf16
BUCK = 4             # table buckets (each <= 32K rows; aligned to NP/4)
MGRP = 8             # blocks per meta-load group
GAT_BUFS = 8
TRUNC_GATHER = True  # -1-padded gathers truncated by per-core count registers
LOCAL_SIM = False    # replace collectives with local copies (TimelineSim)
ABLATE = ""          # "", "nogather" (skip gathers+matmuls), "nomm" (skip M+matmul)
SCL_RING = False     # route self/meta/hb loads via ACT HWDGE ring
IOTA_BF16 = False    # bf16 iota/M-build for DVE 2x
PSA_BUFS = 2
MP_BUFS = 6


def _cdiv(a, b):
    return -(-a // b)


# ----------------------------------------------------------------------------
# host-side preprocessing
# ----------------------------------------------------------------------------

def _prep(x, edge_index, perm, C):
    N, IN = x.shape
    E = edge_index.shape[1]
    assert N % C == 0
    SH = N // C
    NB = _cdiv(SH, 128)
    NP = NB * 128
    NPAD = NP * C
    assert NP % BUCK == 0
    QB = NP // BUCK          # local rows per bucket slice
    BS = C * QB              # rows per bucket table
    assert BS <= 32704

    src = np.asarray(edge_index[0], dtype=np.int64)
    dst = np.asarray(edge_index[1], dtype=np.int64)
    perm = np.asarray(perm, dtype=np.int64)

    deg = (1.0 + np.bincount(dst, minlength=N)).astype(np.float32)
    dinv = (1.0 / np.sqrt(deg)).astype(np.float32)
    enorm = dinv[src] * dinv[dst]
    self_norm = dinv * dinv

    core = dst // SH
    ld = dst - core * SH
    blk = ld >> 7
    dloc = ld & 127
    sr = src // SH
    sl = src - sr * SH
    buck = sl // QB                       # bucket by local row
    sloc = (sr * QB + (sl - buck * QB)).astype(np.int64)  # row in bucket table

    grp = (core * NB + blk) * BUCK + buck
    order = np.argsort(grp, kind="stable")
    grp_s = grp[order]
    cnt = np.bincount(grp, minlength=C * NB * BUCK).reshape(C, NB, BUCK)
    K = _cdiv(cnt, 128).max(axis=0)       # [NB, BUCK] chunks per group
    kflat = K.reshape(-1)
    chunk_base = np.concatenate([[0], np.cumsum(kflat)])
    TOT = int(kflat.sum())

    gstart = np.concatenate([[0], np.cumsum(np.bincount(grp, minlength=C * NB * BUCK))])
    rank = np.arange(E, dtype=np.int64) - gstart[grp_s]
    bu = grp_s % (NB * BUCK)
    slot = chunk_base[bu] * 128 + rank
    core_s = grp_s // (NB * BUCK)

    idxbuf = np.full((C, TOT * 128), -1, np.int16)
    dstbuf = np.zeros((C, TOT * 128), np.float32)
    enobuf = np.zeros((C, TOT * 128), np.float32)
    idxbuf[core_s, slot] = sloc[order].astype(np.int16)
    dstbuf[core_s, slot] = dloc[order].astype(np.float32)
    enobuf[core_s, slot] = enorm[order]
    # a core may have zero edges in a group another core forced (K>0):
    # give the gather one valid index so its count register is never 0.
    zc = (cnt == 0) & (K[None, :, :] > 0)
    if zc.any():
        zci = np.argwhere(zc)
        zslot = chunk_base[zci[:, 1] * BUCK + zci[:, 2]] * 128
        idxbuf[zci[:, 0], zslot] = 0
    gfix = np.maximum(cnt, (K[None, :, :] > 0).astype(cnt.dtype))
    # first WARM_WIN gather windows fetch their full (0-padded) window so
    # that every gather slot's first write covers the whole slot region --
    # afterwards stale bytes are always old gather data (finite).
    warm_left = 16 if TRUNC_GATHER else (NB * BUCK + 1)
    for bb_ in range(NB):
        for uu_ in range(BUCK):
            if warm_left == 0:
                break
            kk = int(K[bb_, uu_])
            if kk == 0:
                continue
            w0 = int(chunk_base[bb_ * BUCK + uu_]) * 128
            win = idxbuf[:, w0:w0 + kk * 128]
            win[win < 0] = 0
            gfix[:, bb_, uu_] = kk * 128
            warm_left -= 1
        if warm_left == 0:
            break

    idx_dev = np.tile(
        idxbuf.reshape(C, TOT, 8, 16).transpose(0, 3, 1, 2).reshape(C, 16, TOT * 8),
        (1, 8, 1),
    )  # [C, 128, TOT*8]
    dst_dev = dstbuf.reshape(C, TOT, 128).transpose(0, 2, 1).copy()
    eno_dev = enobuf.reshape(C, TOT, 128).transpose(0, 2, 1).copy()
    gcnt_dev = np.ascontiguousarray(gfix.reshape(C, NB * BUCK).astype(np.int32))

    sn_pad = np.zeros((C, NP), np.float32)
    sn_pad[:, :SH] = self_norm.reshape(C, SH)
    sn_dev = sn_pad.reshape(C, NB, 128).transpose(0, 2, 1).copy()
    vd_pad = np.zeros((C, NP), np.float32)
    vd_pad[:, :SH] = 1.0
    vd_dev = vd_pad.reshape(C, NB, 128).transpose(0, 2, 1).copy()

    xT_pos = np.zeros((C, IN, NP), np.float32)
    xT_neg = np.zeros((C, IN, NP), np.float32)
    xr = x.reshape(C, SH, IN)
    xn = x[perm].reshape(C, SH, IN)
    for c in range(C):
        xT_pos[c, :, :SH] = xr[c].T
        xT_neg[c, :, :SH] = xn[c].T

    meta = dict(N=N, E=E, IN=IN, SH=SH, NB=NB, NP=NP, NPAD=NPAD, QB=QB,
                BS=BS, TOT=TOT, K=K, chunk_base=chunk_base)
    arrays = dict(idx_dev=idx_dev, dst_dev=dst_dev, eno_dev=eno_dev,
                  gcnt_dev=gcnt_dev, sn_dev=sn_dev, vd_dev=vd_dev,
                  xT_pos=xT_pos, xT_neg=xT_neg)
    return meta, arrays


# ----------------------------------------------------------------------------
# device program
# ----------------------------------------------------------------------------

def _build(meta, HID, OUT, bias1_nz, bias2_nz, bb_val, C):
    N, IN = meta["N"], meta["IN"]
    NB, NP, NPAD = meta["NB"], meta["NP"], meta["NPAD"]
    QB, BS, TOT = meta["QB"], meta["BS"], meta["TOT"]
    K, chunk_base = meta["K"], meta["chunk_base"]
    KI, KH = IN // 128, HID // 128
    assert OUT == 128, "discriminator path assumes OUT == 128"
    F1, F2 = 2 * HID, 2 * OUT
    Kmax = int(K.max())
    GDT = BF16 if GATHER_BF16 else F32   # layer-1 table dtype
    GDT2 = F32                           # layer-2 table dtype (rows stay 1KB)

    nc = bacc.Bacc("TRN2", target_bir_lowering=False, debug=False, num_devices=C,
                   num_swdge_queues=4)

    # inputs
    xtp = nc.dram_tensor("xtp", [IN, NP], F32, kind="ExternalInput")
    xtn = nc.dram_tensor("xtn", [IN, NP], F32, kind="ExternalInput")
    w1 = nc.dram_tensor("w1", [IN, HID], F32, kind="ExternalInput")
    w2 = nc.dram_tensor("w2", [HID, OUT], F32, kind="ExternalInput")
    wbt = nc.dram_tensor("wbt", [OUT, OUT], F32, kind="ExternalInput")
    idx_in = nc.dram_tensor("idx16", [128, TOT * 8], I16, kind="ExternalInput")
    dst_in = nc.dram_tensor("dstl", [128, TOT], F32, kind="ExternalInput")
    eno_in = nc.dram_tensor("enorm", [128, TOT], F32, kind="ExternalInput")
    gcnt_in = nc.dram_tensor("gcnt", [1, NB * BUCK], I32, kind="ExternalInput")
    sn_in = nc.dram_tensor("snorm", [128, NB], F32, kind="ExternalInput")
    vd_in = nc.dram_tensor("valid", [128, NB], F32, kind="ExternalInput")
    b1_in = nc.dram_tensor("b1bc", [128, F1], F32, kind="ExternalInput") if bias1_nz else None
    b2_in = nc.dram_tensor("b2bc", [128, F2], F32, kind="ExternalInput") if bias2_nz else None
    out = nc.dram_tensor("scores", [2, 128, NB], F32, kind="ExternalOutput")

    # internal DRAM
    hw1_sh = nc.dram_tensor("hw1_sh", [NP, F1], GDT, kind="Internal")
    hw2_sh = nc.dram_tensor("hw2_sh", [NP, F2], GDT2, kind="Internal")
    hw1_full = [nc.dram_tensor(f"hw1_full{j}", [BS, F1], GDT, kind="Internal",
                               addr_space="Shared") for j in range(BUCK)]
    hw2_full = [nc.dram_tensor(f"hw2_full{j}", [BS, F2], GDT2, kind="Internal",
                               addr_space="Shared") for j in range(BUCK)]
    h_sh = nc.dram_tensor("h_sh", [NP, F2], F32, kind="Internal")
    cs_in = nc.dram_tensor("cs_in", [128, 1], F32, kind="Internal")
    cs_out = nc.dram_tensor("cs_out", [128, 1], F32, kind="Internal",
                            addr_space="Shared")

    XSPAN = 4

    with tile.TileContext(nc) as tc:
        with tc.tile_pool(name="const", bufs=1) as cp, \
             tc.tile_pool(name="stream", bufs=3) as sp, \
             tc.tile_pool(name="meta", bufs=3) as mpp, \
             tc.tile_pool(name="mpool", bufs=MP_BUFS) as mp, \
             tc.tile_pool(name="gat", bufs=GAT_BUFS) as gp, \
             tc.tile_pool(name="psA", bufs=PSA_BUFS, space="PSUM") as psA, \
             tc.tile_pool(name="psT", bufs=2, space="PSUM") as psT, \
             tc.tile_pool(name="psH", bufs=2, space="PSUM") as psH, \
             tc.tile_pool(name="psC", bufs=1, space="PSUM") as psC:

            def allgather(shard, fulls, F, DTY):
                for j in range(BUCK):
                    if LOCAL_SIM:
                        for i in range(QB // 128):
                            tcp = sp.tile([128, F], DTY, tag="agcopy",
                                          name=f"agc_{shard.name}_{j}_{i}")
                            nc.sync.dma_start(
                                out=tcp[:],
                                in_=shard[j * QB + i * 128:j * QB + (i + 1) * 128, :])
                            nc.sync.dma_start(
                                out=fulls[j][i * 128:(i + 1) * 128, :], in_=tcp[:])
                    else:
                        cc = nc.gpsimd.collective_compute(
                            "AllGather", mybir.AluOpType.bypass,
                            replica_groups=[list(range(C))],
                            ins=[shard[j * QB:(j + 1) * QB, :].opt()],
                            outs=[fulls[j][:, :].opt()])
                        for wi in warm_insts:
                            add_dep_helper(cc.ins, wi, True,
                                           "gather slots must be finite")

            # constants
            ident = cp.tile([128, 128], F32)
            make_identity(nc, ident[:])
            iota = cp.tile([128, 128], BF16 if IOTA_BF16 else F32)
            nc.gpsimd.iota(iota[:], pattern=[[1, 128]], base=0,
                           channel_multiplier=0,
                           allow_small_or_imprecise_dtypes=True)
            w1sb = cp.tile([128, KI, HID], F32)
            for k in range(KI):
                nc.sync.dma_start(out=w1sb[:, k, :], in_=w1[k * 128:(k + 1) * 128, :])
            w2sb = cp.tile([128, KH, OUT], F32)
            for k in range(KH):
                nc.sync.dma_start(out=w2sb[:, k, :], in_=w2[k * 128:(k + 1) * 128, :])
            wbtsb = cp.tile([128, OUT], F32)
            nc.sync.dma_start(out=wbtsb[:], in_=wbt[:, :])
            snsb = cp.tile([128, NB], F32)
            nc.sync.dma_start(out=snsb[:], in_=sn_in[:, :])
            vdsb = cp.tile([128, NB], F32)
            nc.sync.dma_start(out=vdsb[:], in_=vd_in[:, :])
            gcsb = cp.tile([1, NB * BUCK], I32)
            nc.sync.dma_start(out=gcsb[:], in_=gcnt_in[:, :])
            b1sb = b2sb = None
            if bias1_nz:
                b1sb = cp.tile([128, F1], F32)
                nc.sync.dma_start(out=b1sb[:], in_=b1_in[:, :])
            if bias2_nz:
                b2sb = cp.tile([128, F2], F32)
                nc.sync.dma_start(out=b2sb[:], in_=b2_in[:, :])
            sc_pos = cp.tile([128, NB], F32, tag="scp")
            sc_neg = cp.tile([128, NB], F32, tag="scn")

            gregs = [nc.gpsimd.alloc_register(f"gcnt_r{i}") for i in range(4)]

            # warm the gather slots so skipped (-1) rows read finite stale data
            FMAXG = 2 * HID
            warm_insts = []
            for i in range(GAT_BUFS):
                gw = gp.tile([128, Kmax, FMAXG], BF16 if GATHER_BF16 else F32,
                             tag="gat", name=f"gwarm{i}")
                warm_insts.append(nc.vector.memset(gw[:], 0.0).ins)

            # ---------------- phase A: hw1 = x @ W1 (pos|neg) ----------------
            for sb0 in range(0, NB, XSPAN):
                span = min(XSPAN, NB - sb0)
                xp = sp.tile([128, KI, XSPAN * 128], F32, tag="xtp")
                xn_t = sp.tile([128, KI, XSPAN * 128], F32, tag="xtn")
                for k in range(KI):
                    nc.sync.dma_start(
                        out=xp[:, k, :span * 128],
                        in_=xtp[k * 128:(k + 1) * 128, sb0 * 128:(sb0 + span) * 128])
                    nc.sync.dma_start(
                        out=xn_t[:, k, :span * 128],
                        in_=xtn[k * 128:(k + 1) * 128, sb0 * 128:(sb0 + span) * 128])
                for j in range(span):
                    nb_ = sb0 + j
                    pa = psA.tile([128, F1], F32, tag="agg", space="PSUM")
                    for k in range(KI):
                        nc.tensor.matmul(
                            out=pa[:, 0:HID],
                            lhsT=xp[:, k, j * 128:(j + 1) * 128],
                            rhs=w1sb[:, k, :],
                            start=(k == 0), stop=(k == KI - 1))
                    for k in range(KI):
                        nc.tensor.matmul(
                            out=pa[:, HID:F1],
                            lhsT=xn_t[:, k, j * 128:(j + 1) * 128],
                            rhs=w1sb[:, k, :],
                            start=(k == 0), stop=(k == KI - 1))
                    hw1sb = sp.tile([128, F1], GDT, tag="hw1sb")
                    nc.vector.tensor_copy(out=hw1sb[:], in_=pa[:])
                    nc.sync.dma_start(out=hw1_sh[nb_ * 128:(nb_ + 1) * 128, :],
                                      in_=hw1sb[:])

            allgather(hw1_sh, hw1_full, F1, GDT)

            # ---------------- aggregation layers ----------------
            def agg_layer(layer):
                F = F1 if layer == 1 else F2
                DTY = GDT if layer == 1 else GDT2
                fulls = hw1_full if layer == 1 else hw2_full
                shard = hw1_sh if layer == 1 else hw2_sh
                bsb = b1sb if layer == 1 else b2sb
                dl = en = ix = None
                g0 = 0
                post_prev = None
                for b in range(NB):
                    kb = 0 if ABLATE == "nogather" else int(K[b].sum())
                    co = int(chunk_base[b * BUCK])
                    if b % MGRP == 0:
                        g0 = b
                        ge = min(NB, b + MGRP)
                        gc0 = int(chunk_base[g0 * BUCK])
                        gcols = int(chunk_base[ge * BUCK]) - gc0
                        if gcols > 0:
                            dl = mpp.tile([128, gcols], F32, tag="dl",
                                          name=f"dl{layer}_{b}")
                            en = mpp.tile([128, gcols], F32, tag="en",
                                          name=f"en{layer}_{b}")
                            ix = mpp.tile([128, gcols * 8], I16, tag="ix",
                                          name=f"ix{layer}_{b}")
                            meng = nc.scalar if SCL_RING else nc.sync
                            meng.dma_start(out=dl[:], in_=dst_in[:, gc0:gc0 + gcols])
                            meng.dma_start(out=en[:], in_=eno_in[:, gc0:gc0 + gcols])
                            meng.dma_start(out=ix[:],
                                           in_=idx_in[:, gc0 * 8:(gc0 + gcols) * 8])
                    lo = co - int(chunk_base[g0 * BUCK])
                    # issue self-row load early
                    selfr = sp.tile([128, F], DTY, tag="selfr",
                                    name=f"sf{layer}_{b}")
                    (nc.scalar if SCL_RING else nc.sync).dma_start(
                        out=selfr[:], in_=shard[b * 128:(b + 1) * 128, :])
                    ps_agg = None
                    if kb > 0:
                        gts = []
                        t0 = 0
                        for u in range(BUCK):
                            ku = int(K[b, u])
                            if ku == 0:
                                gts.append(None)
                                continue
                            gt = gp.tile([128, Kmax, F], DTY, tag="gat",
                                         name=f"gt{layer}_{b}_{u}")
                            if TRUNC_GATHER:
                                reg = gregs[u % len(gregs)]
                                nc.gpsimd.reg_load(
                                    reg, gcsb[0:1, b * BUCK + u:b * BUCK + u + 1])
                            else:
                                reg = ku * 128
                            nc.gpsimd.dma_gather(
                                out_ap=gt[:, :ku, :],
                                in_ap=fulls[u][:, :],
                                idxs_ap=ix[:, (lo + t0) * 8:(lo + t0 + ku) * 8],
                                num_idxs=ku * 128,
                                num_idxs_reg=reg,
                                elem_size=F,
                                single_packet=(ku * 128 <= 1024),
                                queue_num=(b * BUCK + u) % 4)
                            gts.append(gt)
                            t0 += ku
                        ps_agg = psA.tile([128, F1], F32, tag="agg", space="PSUM")
                        t = 0
                        for u in range(BUCK):
                            ku = 0 if ABLATE == "nomm" else int(K[b, u])
                            for j in range(ku):
                                m = mp.tile([128, 128], DTY, tag="m",
                                            name=f"m{layer}_{b}_{t}")
                                nc.vector.tensor_scalar(
                                    out=m[:], in0=iota[:],
                                    scalar1=dl[:, lo + t:lo + t + 1],
                                    scalar2=en[:, lo + t:lo + t + 1],
                                    op0=mybir.AluOpType.is_equal,
                                    op1=mybir.AluOpType.mult)
                                nc.tensor.matmul(
                                    out=ps_agg[:, :F], lhsT=m[:], rhs=gts[u][:, j, :],
                                    start=(t == 0), stop=(t == kb - 1))
                                t += 1

                    def make_post(b=b, kb=kb, ps_agg=ps_agg, selfr=selfr):
                        def post():
                            hout = sp.tile([128, F], F32, tag="hout",
                                           name=f"ho{layer}_{b}")
                            if kb > 0 and ABLATE != "nomm":
                                nc.vector.scalar_tensor_tensor(
                                    out=hout[:], in0=selfr[:],
                                    scalar=snsb[:, b:b + 1],
                                    in1=ps_agg[:, :F],
                                    op0=mybir.AluOpType.mult,
                                    op1=mybir.AluOpType.add)
                            else:
                                nc.vector.tensor_scalar(
                                    out=hout[:], in0=selfr[:],
                                    scalar1=snsb[:, b:b + 1],
                                    scalar2=None, op0=mybir.AluOpType.mult)
                            if bsb is not None:
                                nc.vector.tensor_tensor(
                                    out=hout[:], in0=hout[:], in1=bsb[:],
                                    op=mybir.AluOpType.add)
                            if layer == 1:
                                nc.vector.tensor_scalar(
                                    out=hout[:], in0=hout[:], scalar1=0.0,
                                    scalar2=None, op0=mybir.AluOpType.max)
                                ps_tp = psT.tile([128, F1], F32, tag="tp",
                                                 space="PSUM", name=f"tp{layer}_{b}")
                                for k in range(2 * KH):
                                    nc.tensor.transpose(
                                        out=ps_tp[:, k * 128:(k + 1) * 128],
                                        in_=hout[:, k * 128:(k + 1) * 128],
                                        identity=ident[:])
                                ts = sp.tile([128, F1], F32, tag="ts",
                                             name=f"ts{layer}_{b}")
                                nc.scalar.copy(out=ts[:], in_=ps_tp[:])
                                ps_h2 = psH.tile([128, F2], F32, tag="h2",
                                                 space="PSUM", name=f"h2{layer}_{b}")
                                for k in range(KH):
                                    nc.tensor.matmul(
                                        out=ps_h2[:, 0:OUT],
                                        lhsT=ts[:, k * 128:(k + 1) * 128],
                                        rhs=w2sb[:, k, :],
                                        start=(k == 0), stop=(k == KH - 1))
                                for k in range(KH):
                                    nc.tensor.matmul(
                                        out=ps_h2[:, OUT:F2],
                                        lhsT=ts[:, (KH + k) * 128:(KH + k + 1) * 128],
                                        rhs=w2sb[:, k, :],
                                        start=(k == 0), stop=(k == KH - 1))
                                hw2sb = sp.tile([128, F2], GDT2, tag="hw2sb",
                                                name=f"hw2sb{layer}_{b}")
                                nc.scalar.copy(out=hw2sb[:], in_=ps_h2[:])
                                nc.sync.dma_start(
                                    out=hw2_sh[b * 128:(b + 1) * 128, :],
                                    in_=hw2sb[:])
                            else:
                                if b == 0:
                                    ps_cs = psC.tile([128, 1], F32, tag="cs",
                                                     space="PSUM")
                                    agg_layer.cs = ps_cs
                                else:
                                    ps_cs = agg_layer.cs
                                nc.tensor.matmul(
                                    out=ps_cs[:], lhsT=hout[:, 0:OUT],
                                    rhs=vdsb[:, b:b + 1],
                                    start=(b == 0), stop=(b == NB - 1),
                                    skip_group_check=True)
                                nc.sync.dma_start(
                                    out=h_sh[b * 128:(b + 1) * 128, :],
                                    in_=hout[:])
                        return post

                    if post_prev is not None:
                        post_prev()
                    post_prev = make_post()
                post_prev()

            agg_layer(1)
            allgather(hw2_sh, hw2_full, F2, GDT2)
            agg_layer(2)

            # ---------------- summary s and v = Wb @ s ----------------
            cssb = sp.tile([128, 1], F32, tag="cssb")
            nc.vector.tensor_copy(out=cssb[:], in_=agg_layer.cs[:])
            nc.sync.dma_start(out=cs_in[:, :], in_=cssb[:])
            if LOCAL_SIM:
                nc.sync.dma_start(out=cs_out[:, :], in_=cssb[:])
            else:
                nc.gpsimd.collective_compute(
                    "AllReduce", mybir.AluOpType.add,
                    replica_groups=[list(range(C))],
                    ins=[cs_in[:, :].opt()], outs=[cs_out[:, :].opt()])
            csr = sp.tile([128, 1], F32, tag="csr")
            nc.sync.dma_start(out=csr[:], in_=cs_out[:, :])
            ssb = sp.tile([128, 1], F32, tag="ssb")
            nc.scalar.activation(out=ssb[:], in_=csr[:],
                                 func=mybir.ActivationFunctionType.Sigmoid,
                                 scale=1.0 / N)
            ps_v = psC.tile([128, 1], F32, tag="cs", space="PSUM")
            nc.tensor.matmul(out=ps_v[:], lhsT=wbtsb[:], rhs=ssb[:],
                             start=True, stop=True)
            vsb = sp.tile([128, 1], F32, tag="vsb")
            nc.vector.tensor_copy(out=vsb[:], in_=ps_v[:])

            # ---------------- scores ----------------
            for b in range(NB):
                hb = sp.tile([128, F2], F32, tag="hb")
                (nc.scalar if SCL_RING else nc.sync).dma_start(
                    out=hb[:], in_=h_sh[b * 128:(b + 1) * 128, :])
                ps_tp = psT.tile([128, F1], F32, tag="tp", space="PSUM")
                for k in range(2):
                    nc.tensor.transpose(
                        out=ps_tp[:, k * 128:(k + 1) * 128],
                        in_=hb[:, k * OUT:k * OUT + 128],
                        identity=ident[:])
                ts2 = sp.tile([128, F2], F32, tag="ts2")
                nc.vector.tensor_copy(out=ts2[:], in_=ps_tp[:, :F2])
                ps_sc = psH.tile([128, F2], F32, tag="h2", space="PSUM")
                nc.tensor.matmul(out=ps_sc[:, 0:1], lhsT=ts2[:, 0:128],
                                 rhs=vsb[:], start=True, stop=True)
                nc.tensor.matmul(out=ps_sc[:, 1:2], lhsT=ts2[:, 128:256],
                                 rhs=vsb[:], start=True, stop=True)
                nc.vector.tensor_scalar(
                    out=sc_pos[:, b:b + 1], in0=ps_sc[:, 0:1],
                    scalar1=float(bb_val), scalar2=None, op0=mybir.AluOpType.add)
                nc.vector.tensor_scalar(
                    out=sc_neg[:, b:b + 1], in0=ps_sc[:, 1:2],
                    scalar1=float(bb_val), scalar2=None, op0=mybir.AluOpType.add)
            nc.sync.dma_start(out=out[0, :, :], in_=sc_pos[:])
            nc.sync.dma_start(out=out[1, :, :], in_=sc_neg[:])

    nc.compile()
    return nc


# ----------------------------------------------------------------------------
# entry point
# ----------------------------------------------------------------------------

_CACHE = {}


def _get_program(meta, HID, OUT, bias1_nz, bias2_nz, bb_val, C):
    key = (meta["N"], meta["E"], meta["IN"], HID, OUT, bias1_nz, bias2_nz,
           float(bb_val), C, meta["TOT"], meta["K"].tobytes())
    if key not in _CACHE:
        _CACHE[key] = _build(meta, HID, OUT, bias1_nz, bias2_nz, bb_val, C)
    return _CACHE[key]


def _make_in_maps(meta, arrs, W1, b1, W2, b2, Wb, C, bias1_nz, bias2_nz):
    in_maps = []
    for c in range(C):
        m = {
            "xtp": arrs["xT_pos"][c], "xtn": arrs["xT_neg"][c],
            "w1": W1, "w2": W2, "wbt": np.ascontiguousarray(Wb.T),
            "idx16": arrs["idx_dev"][c], "dstl": arrs["dst_dev"][c],
            "enorm": arrs["eno_dev"][c], "gcnt": arrs["gcnt_dev"][c][None, :],
            "snorm": arrs["sn_dev"][c], "valid": arrs["vd_dev"][c],
        }
        if bias1_nz:
            m["b1bc"] = np.tile(np.concatenate([b1, b1])[None, :], (128, 1))
        if bias2_nz:
            m["b2bc"] = np.tile(np.concatenate([b2, b2])[None, :], (128, 1))
        in_maps.append(m)
    return in_maps


def kernel(x, edge_index, perm, W1, b1, W2, b2, Wb, bb):
    C = 8
    x = np.asarray(x, np.float32)
    W1 = np.asarray(W1, np.float32)
    W2 = np.asarray(W2, np.float32)
    Wb = np.asarray(Wb, np.float32)
    b1 = np.asarray(b1, np.float32)
    b2 = np.asarray(b2, np.float32)
    bb_val = float(np.asarray(bb).reshape(-1)[0])
    N = x.shape[0]
    HID = W1.shape[1]
    OUT = W2.shape[1]

    meta, arrs = _prep(x, edge_index, perm, C)
    bias1_nz = bool(np.any(b1))
    bias2_nz = bool(np.any(b2))
    nc = _get_program(meta, HID, OUT, bias1_nz, bias2_nz, bb_val, C)
    in_maps = _make_in_maps(meta, arrs, W1, b1, W2, b2, Wb, C, bias1_nz, bias2_nz)

    res = bass_utils.run_bass_kernel_spmd(nc, in_maps, core_ids=list(range(C)))

    SH, NB = meta["SH"], meta["NB"]
    pos = np.empty((N, 1), np.float32)
    neg = np.empty((N, 1), np.float32)
    for c in range(C):
        sc = res.results[c]["scores"]
        pos[c * SH:(c + 1) * SH, 0] = sc[0].T.reshape(-1)[:SH]
        neg[c * SH:(c + 1) * SH, 0] = sc[1].T.reshape(-1)[:SH]
    return pos, neg



# revision 2
# speedup vs baseline: 1.2117x; 1.2117x over previous
"""DGI (2-layer GCN encoder + bilinear discriminator) on 8 TRN2 NeuronCores.

Sharding: nodes (and edges by destination) across 8 cores. Per layer, each
core computes its shard of h @ W (pos|neg fused on the feature axis), the
full feature table is AllGathered in 4 bucket slices (each <=32K rows so
int16 gather indices work, and so aggregation overlaps the collectives),
then the segment-sum aggregation for the core's destination shard runs as
dma_gather row gathers + one-DVE-op selection matrices + PSUM-accumulated
matmuls. Gathers are truncated per-core via -1 index padding and a
num_idxs register loaded from an input, so padding costs no HBM traffic.
The graph summary is a masked column-sum + AllReduce; scores come from a
PE transpose + matvec against Wb @ sigmoid(mean H).
"""
import sys
sys.path.insert(0, "/opt/trn_rl_repo")

import numpy as np
import concourse.bass as bass
import concourse.bacc as bacc
import concourse.tile as tile
from concourse import bass_utils, mybir
from concourse.masks import make_identity
from concourse.tile import add_dep_helper

F32 = mybir.dt.float32
BF16 = mybir.dt.bfloat16
I16 = mybir.dt.int16
I32 = mybir.dt.int32

GATHER_BF16 = True   # feature tables gathered by edges + collectives in bf16
BUCK = 4             # table buckets (each <= 32K rows; aligned to NP/4)
MGRP = 8             # blocks per meta-load group
GAT_BUFS = 8
TRUNC_GATHER = True  # -1-padded gathers truncated by per-core count registers
LOCAL_SIM = False    # replace collectives with local copies (TimelineSim)
ABLATE = ""          # "", "nogather" (skip gathers+matmuls), "nomm" (skip M+matmul)
SCL_RING = False     # route self/meta/hb loads via ACT HWDGE ring
IOTA_BF16 = False    # bf16 iota/M-build for DVE 2x
PSA_BUFS = 2
MP_BUFS = 6


def _cdiv(a, b):
    return -(-a // b)


# ----------------------------------------------------------------------------
# host-side preprocessing
# ----------------------------------------------------------------------------

def _prep(x, edge_index, perm, C):
    N, IN = x.shape
    E = edge_index.shape[1]
    assert N % C == 0
    SH = N // C
    NB = _cdiv(SH, 128)
    NP = NB * 128
    NPAD = NP * C
    assert NP % BUCK == 0
    QB = NP // BUCK          # local rows per bucket slice
    BS = C * QB              # rows per bucket table
    assert BS <= 32704

    src = np.asarray(edge_index[0], dtype=np.int64)
    dst = np.asarray(edge_index[1], dtype=np.int64)
    perm = np.asarray(perm, dtype=np.int64)

    deg = (1.0 + np.bincount(dst, minlength=N)).astype(np.float32)
    dinv = (1.0 / np.sqrt(deg)).astype(np.float32)
    enorm = dinv[src] * dinv[dst]
    self_norm = dinv * dinv

    core = dst // SH
    ld = dst - core * SH
    blk = ld >> 7
    dloc = ld & 127
    sr = src // SH
    sl = src - sr * SH
    buck = sl // QB                       # bucket by local row
    sloc = (sr * QB + (sl - buck * QB)).astype(np.int64)  # row in bucket table

    grp = (core * NB + blk) * BUCK + buck
    order = np.argsort(grp, kind="stable")
    grp_s = grp[order]
    cnt = np.bincount(grp, minlength=C * NB * BUCK).reshape(C, NB, BUCK)
    K = _cdiv(cnt, 128).max(axis=0)       # [NB, BUCK] chunks per group
    kflat = K.reshape(-1)
    chunk_base = np.concatenate([[0], np.cumsum(kflat)])
    TOT = int(kflat.sum())

    gstart = np.concatenate([[0], np.cumsum(np.bincount(grp, minlength=C * NB * BUCK))])
    rank = np.arange(E, dtype=np.int64) - gstart[grp_s]
    bu = grp_s % (NB * BUCK)
    slot = chunk_base[bu] * 128 + rank
    core_s = grp_s // (NB * BUCK)

    idxbuf = np.full((C, TOT * 128), -1, np.int16)
    dstbuf = np.zeros((C, TOT * 128), np.float32)
    enobuf = np.zeros((C, TOT * 128), np.float32)
    idxbuf[core_s, slot] = sloc[order].astype(np.int16)
    dstbuf[core_s, slot] = dloc[order].astype(np.float32)
    enobuf[core_s, slot] = enorm[order]
    # a core may have zero edges in a group another core forced (K>0):
    # give the gather one valid index so its count register is never 0.
    zc = (cnt == 0) & (K[None, :, :] > 0)
    if zc.any():
        zci = np.argwhere(zc)
        zslot = chunk_base[zci[:, 1] * BUCK + zci[:, 2]] * 128
        idxbuf[zci[:, 0], zslot] = 0
    gfix = np.maximum(cnt, (K[None, :, :] > 0).astype(cnt.dtype))
    # first WARM_WIN gather windows fetch their full (0-padded) window so
    # that every gather slot's first write covers the whole slot region --
    # afterwards stale bytes are always old gather data (finite).
    warm_left = 16 if TRUNC_GATHER else (NB * BUCK + 1)
    for bb_ in range(NB):
        for uu_ in range(BUCK):
            if warm_left == 0:
                break
            kk = int(K[bb_, uu_])
            if kk == 0:
                continue
            w0 = int(chunk_base[bb_ * BUCK + uu_]) * 128
            win = idxbuf[:, w0:w0 + kk * 128]
            win[win < 0] = 0
            gfix[:, bb_, uu_] = kk * 128
            warm_left -= 1
        if warm_left == 0:
            break

    idx_dev = np.tile(
        idxbuf.reshape(C, TOT, 8, 16).transpose(0, 3, 1, 2).reshape(C, 16, TOT * 8),
        (1, 8, 1),
    )  # [C, 128, TOT*8]
    dst_dev = dstbuf.reshape(C, TOT, 128).transpose(0, 2, 1).copy()
    eno_dev = enobuf.reshape(C, TOT, 128).transpose(0, 2, 1).copy()
    gcnt_dev = np.ascontiguousarray(gfix.reshape(C, NB * BUCK).astype(np.int32))

    sn_pad = np.zeros((C, NP), np.float32)
    sn_pad[:, :SH] = self_norm.reshape(C, SH)
    sn_dev = sn_pad.reshape(C, NB, 128).transpose(0, 2, 1).copy()
    vd_pad = np.zeros((C, NP), np.float32)
    vd_pad[:, :SH] = 1.0
    vd_dev = vd_pad.reshape(C, NB, 128).transpose(0, 2, 1).copy()

    xT_pos = np.zeros((C, IN, NP), np.float32)
    xT_neg = np.zeros((C, IN, NP), np.float32)
    xr = x.reshape(C, SH, IN)
    xn = x[perm].reshape(C, SH, IN)
    for c in range(C):
        xT_pos[c, :, :SH] = xr[c].T
        xT_neg[c, :, :SH] = xn[c].T

    meta = dict(N=N, E=E, IN=IN, SH=SH, NB=NB, NP=NP, NPAD=NPAD, QB=QB,
                BS=BS, TOT=TOT, K=K, chunk_base=chunk_base)
    arrays = dict(idx_dev=idx_dev, dst_dev=dst_dev, eno_dev=eno_dev,
                  gcnt_dev=gcnt_dev, sn_dev=sn_dev, vd_dev=vd_dev,
                  xT_pos=xT_pos, xT_neg=xT_neg)
    return meta, arrays


# ----------------------------------------------------------------------------
# device program
# ----------------------------------------------------------------------------

def _build(meta, HID, OUT, bias1_nz, bias2_nz, bb_val, C):
    N, IN = meta["N"], meta["IN"]
    NB, NP, NPAD = meta["NB"], meta["NP"], meta["NPAD"]
    QB, BS, TOT = meta["QB"], meta["BS"], meta["TOT"]
    K, chunk_base = meta["K"], meta["chunk_base"]
    KI, KH = IN // 128, HID // 128
    assert OUT == 128, "discriminator path assumes OUT == 128"
    F1, F2 = 2 * HID, 2 * OUT
    Kmax = int(K.max())
    GDT = BF16 if GATHER_BF16 else F32   # layer-1 table dtype
    GDT2 = BF16 if GATHER_BF16 else F32  # layer-2 table dtype (512B rows)

    nc = bacc.Bacc("TRN2", target_bir_lowering=False, debug=False, num_devices=C,
                   num_swdge_queues=4)

    # inputs
    xtp = nc.dram_tensor("xtp", [IN, NP], F32, kind="ExternalInput")
    xtn = nc.dram_tensor("xtn", [IN, NP], F32, kind="ExternalInput")
    w1 = nc.dram_tensor("w1", [IN, HID], F32, kind="ExternalInput")
    w2 = nc.dram_tensor("w2", [HID, OUT], F32, kind="ExternalInput")
    wbt = nc.dram_tensor("wbt", [OUT, OUT], F32, kind="ExternalInput")
    idx_in = nc.dram_tensor("idx16", [128, TOT * 8], I16, kind="ExternalInput")
    dst_in = nc.dram_tensor("dstl", [128, TOT], F32, kind="ExternalInput")
    eno_in = nc.dram_tensor("enorm", [128, TOT], F32, kind="ExternalInput")
    gcnt_in = nc.dram_tensor("gcnt", [1, NB * BUCK], I32, kind="ExternalInput")
    sn_in = nc.dram_tensor("snorm", [128, NB], F32, kind="ExternalInput")
    vd_in = nc.dram_tensor("valid", [128, NB], F32, kind="ExternalInput")
    b1_in = nc.dram_tensor("b1bc", [128, F1], F32, kind="ExternalInput") if bias1_nz else None
    b2_in = nc.dram_tensor("b2bc", [128, F2], F32, kind="ExternalInput") if bias2_nz else None
    out = nc.dram_tensor("scores", [2, 128, NB], F32, kind="ExternalOutput")

    # internal DRAM
    hw1_sh = nc.dram_tensor("hw1_sh", [NP, F1], GDT, kind="Internal")
    hw2_sh = nc.dram_tensor("hw2_sh", [NP, F2], GDT2, kind="Internal")
    hw1_full = [nc.dram_tensor(f"hw1_full{j}", [BS, F1], GDT, kind="Internal",
                               addr_space="Shared") for j in range(BUCK)]
    hw2_full = [nc.dram_tensor(f"hw2_full{j}", [BS, F2], GDT2, kind="Internal",
                               addr_space="Shared") for j in range(BUCK)]
    h_sh = nc.dram_tensor("h_sh", [NP, F2], F32, kind="Internal")
    cs_in = nc.dram_tensor("cs_in", [128, 1], F32, kind="Internal")
    cs_out = nc.dram_tensor("cs_out", [128, 1], F32, kind="Internal",
                            addr_space="Shared")

    XSPAN = 4

    with tile.TileContext(nc) as tc:
        with tc.tile_pool(name="const", bufs=1) as cp, \
             tc.tile_pool(name="stream", bufs=3) as sp, \
             tc.tile_pool(name="meta", bufs=3) as mpp, \
             tc.tile_pool(name="mpool", bufs=MP_BUFS) as mp, \
             tc.tile_pool(name="gat", bufs=GAT_BUFS) as gp, \
             tc.tile_pool(name="psA", bufs=PSA_BUFS, space="PSUM") as psA, \
             tc.tile_pool(name="psT", bufs=2, space="PSUM") as psT, \
             tc.tile_pool(name="psH", bufs=2, space="PSUM") as psH, \
             tc.tile_pool(name="psC", bufs=1, space="PSUM") as psC:

            def allgather(shard, fulls, F, DTY):
                for j in range(BUCK):
                    if LOCAL_SIM:
                        for i in range(QB // 128):
                            tcp = sp.tile([128, F], DTY, tag="agcopy",
                                          name=f"agc_{shard.name}_{j}_{i}")
                            nc.sync.dma_start(
                                out=tcp[:],
                                in_=shard[j * QB + i * 128:j * QB + (i + 1) * 128, :])
                            nc.sync.dma_start(
                                out=fulls[j][i * 128:(i + 1) * 128, :], in_=tcp[:])
                    else:
                        cc = nc.gpsimd.collective_compute(
                            "AllGather", mybir.AluOpType.bypass,
                            replica_groups=[list(range(C))],
                            ins=[shard[j * QB:(j + 1) * QB, :].opt()],
                            outs=[fulls[j][:, :].opt()])
                        for wi in warm_insts:
                            add_dep_helper(cc.ins, wi, True,
                                           "gather slots must be finite")

            # constants
            ident = cp.tile([128, 128], F32)
            make_identity(nc, ident[:])
            iota = cp.tile([128, 128], BF16 if IOTA_BF16 else F32)
            nc.gpsimd.iota(iota[:], pattern=[[1, 128]], base=0,
                           channel_multiplier=0,
                           allow_small_or_imprecise_dtypes=True)
            w1sb = cp.tile([128, KI, HID], F32)
            for k in range(KI):
                nc.sync.dma_start(out=w1sb[:, k, :], in_=w1[k * 128:(k + 1) * 128, :])
            w2sb = cp.tile([128, KH, OUT], F32)
            for k in range(KH):
                nc.sync.dma_start(out=w2sb[:, k, :], in_=w2[k * 128:(k + 1) * 128, :])
            wbtsb = cp.tile([128, OUT], F32)
            nc.sync.dma_start(out=wbtsb[:], in_=wbt[:, :])
            snsb = cp.tile([128, NB], F32)
            nc.sync.dma_start(out=snsb[:], in_=sn_in[:, :])
            vdsb = cp.tile([128, NB], F32)
            nc.sync.dma_start(out=vdsb[:], in_=vd_in[:, :])
            gcsb = cp.tile([1, NB * BUCK], I32)
            nc.sync.dma_start(out=gcsb[:], in_=gcnt_in[:, :])
            b1sb = b2sb = None
            if bias1_nz:
                b1sb = cp.tile([128, F1], F32)
                nc.sync.dma_start(out=b1sb[:], in_=b1_in[:, :])
            if bias2_nz:
                b2sb = cp.tile([128, F2], F32)
                nc.sync.dma_start(out=b2sb[:], in_=b2_in[:, :])
            sc_pos = cp.tile([128, NB], F32, tag="scp")
            sc_neg = cp.tile([128, NB], F32, tag="scn")

            gregs = [nc.gpsimd.alloc_register(f"gcnt_r{i}") for i in range(4)]

            # warm the gather slots so skipped (-1) rows read finite stale data
            FMAXG = 2 * HID
            warm_insts = []
            for i in range(GAT_BUFS):
                gw = gp.tile([128, Kmax, FMAXG], BF16 if GATHER_BF16 else F32,
                             tag="gat", name=f"gwarm{i}")
                warm_insts.append(nc.vector.memset(gw[:], 0.0).ins)

            # ---------------- phase A: hw1 = x @ W1 (pos|neg) ----------------
            for sb0 in range(0, NB, XSPAN):
                span = min(XSPAN, NB - sb0)
                xp = sp.tile([128, KI, XSPAN * 128], F32, tag="xtp")
                xn_t = sp.tile([128, KI, XSPAN * 128], F32, tag="xtn")
                for k in range(KI):
                    nc.sync.dma_start(
                        out=xp[:, k, :span * 128],
                        in_=xtp[k * 128:(k + 1) * 128, sb0 * 128:(sb0 + span) * 128])
                    nc.sync.dma_start(
                        out=xn_t[:, k, :span * 128],
                        in_=xtn[k * 128:(k + 1) * 128, sb0 * 128:(sb0 + span) * 128])
                for j in range(span):
                    nb_ = sb0 + j
                    pa = psA.tile([128, F1], F32, tag="agg", space="PSUM")
                    for k in range(KI):
                        nc.tensor.matmul(
                            out=pa[:, 0:HID],
                            lhsT=xp[:, k, j * 128:(j + 1) * 128],
                            rhs=w1sb[:, k, :],
                            start=(k == 0), stop=(k == KI - 1))
                    for k in range(KI):
                        nc.tensor.matmul(
                            out=pa[:, HID:F1],
                            lhsT=xn_t[:, k, j * 128:(j + 1) * 128],
                            rhs=w1sb[:, k, :],
                            start=(k == 0), stop=(k == KI - 1))
                    hw1sb = sp.tile([128, F1], GDT, tag="hw1sb")
                    nc.vector.tensor_copy(out=hw1sb[:], in_=pa[:])
                    nc.sync.dma_start(out=hw1_sh[nb_ * 128:(nb_ + 1) * 128, :],
                                      in_=hw1sb[:])

            allgather(hw1_sh, hw1_full, F1, GDT)

            # ---------------- aggregation layers ----------------
            def agg_layer(layer):
                F = F1 if layer == 1 else F2
                DTY = GDT if layer == 1 else GDT2
                fulls = hw1_full if layer == 1 else hw2_full
                shard = hw1_sh if layer == 1 else hw2_sh
                bsb = b1sb if layer == 1 else b2sb
                dl = en = ix = None
                g0 = 0
                post_prev = None
                for b in range(NB):
                    kb = 0 if ABLATE == "nogather" else int(K[b].sum())
                    co = int(chunk_base[b * BUCK])
                    if b % MGRP == 0:
                        g0 = b
                        ge = min(NB, b + MGRP)
                        gc0 = int(chunk_base[g0 * BUCK])
                        gcols = int(chunk_base[ge * BUCK]) - gc0
                        if gcols > 0:
                            dl = mpp.tile([128, gcols], F32, tag="dl",
                                          name=f"dl{layer}_{b}")
                            en = mpp.tile([128, gcols], F32, tag="en",
                                          name=f"en{layer}_{b}")
                            ix = mpp.tile([128, gcols * 8], I16, tag="ix",
                                          name=f"ix{layer}_{b}")
                            meng = nc.scalar if SCL_RING else nc.sync
                            meng.dma_start(out=dl[:], in_=dst_in[:, gc0:gc0 + gcols])
                            meng.dma_start(out=en[:], in_=eno_in[:, gc0:gc0 + gcols])
                            meng.dma_start(out=ix[:],
                                           in_=idx_in[:, gc0 * 8:(gc0 + gcols) * 8])
                    lo = co - int(chunk_base[g0 * BUCK])
                    # issue self-row load early
                    selfr = sp.tile([128, F], DTY, tag="selfr",
                                    name=f"sf{layer}_{b}")
                    (nc.scalar if SCL_RING else nc.sync).dma_start(
                        out=selfr[:], in_=shard[b * 128:(b + 1) * 128, :])
                    ps_agg = None
                    if kb > 0:
                        gts = []
                        t0 = 0
                        for u in range(BUCK):
                            ku = int(K[b, u])
                            if ku == 0:
                                gts.append(None)
                                continue
                            gt = gp.tile([128, Kmax, F], DTY, tag="gat",
                                         name=f"gt{layer}_{b}_{u}")
                            if TRUNC_GATHER:
                                reg = gregs[u % len(gregs)]
                                nc.gpsimd.reg_load(
                                    reg, gcsb[0:1, b * BUCK + u:b * BUCK + u + 1])
                            else:
                                reg = ku * 128
                            nc.gpsimd.dma_gather(
                                out_ap=gt[:, :ku, :],
                                in_ap=fulls[u][:, :],
                                idxs_ap=ix[:, (lo + t0) * 8:(lo + t0 + ku) * 8],
                                num_idxs=ku * 128,
                                num_idxs_reg=reg,
                                elem_size=F,
                                single_packet=(ku * 128 <= 1024),
                                queue_num=(b * BUCK + u) % 4)
                            gts.append(gt)
                            t0 += ku
                        ps_agg = psA.tile([128, F1], F32, tag="agg", space="PSUM")
                        t = 0
                        for u in range(BUCK):
                            ku = 0 if ABLATE == "nomm" else int(K[b, u])
                            for j in range(ku):
                                m = mp.tile([128, 128], DTY, tag="m",
                                            name=f"m{layer}_{b}_{t}")
                                nc.vector.tensor_scalar(
                                    out=m[:], in0=iota[:],
                                    scalar1=dl[:, lo + t:lo + t + 1],
                                    scalar2=en[:, lo + t:lo + t + 1],
                                    op0=mybir.AluOpType.is_equal,
                                    op1=mybir.AluOpType.mult)
                                nc.tensor.matmul(
                                    out=ps_agg[:, :F], lhsT=m[:], rhs=gts[u][:, j, :],
                                    start=(t == 0), stop=(t == kb - 1))
                                t += 1

                    def make_post(b=b, kb=kb, ps_agg=ps_agg, selfr=selfr):
                        def post():
                            hout = sp.tile([128, F], F32, tag="hout",
                                           name=f"ho{layer}_{b}")
                            if kb > 0 and ABLATE != "nomm":
                                nc.vector.scalar_tensor_tensor(
                                    out=hout[:], in0=selfr[:],
                                    scalar=snsb[:, b:b + 1],
                                    in1=ps_agg[:, :F],
                                    op0=mybir.AluOpType.mult,
                                    op1=mybir.AluOpType.add)
                            else:
                                nc.vector.tensor_scalar(
                                    out=hout[:], in0=selfr[:],
                                    scalar1=snsb[:, b:b + 1],
                                    scalar2=None, op0=mybir.AluOpType.mult)
                            if bsb is not None:
                                nc.vector.tensor_tensor(
                                    out=hout[:], in0=hout[:], in1=bsb[:],
                                    op=mybir.AluOpType.add)
                            if layer == 1:
                                nc.vector.tensor_scalar(
                                    out=hout[:], in0=hout[:], scalar1=0.0,
                                    scalar2=None, op0=mybir.AluOpType.max)
                                ps_tp = psT.tile([128, F1], F32, tag="tp",
                                                 space="PSUM", name=f"tp{layer}_{b}")
                                for k in range(2 * KH):
                                    nc.tensor.transpose(
                                        out=ps_tp[:, k * 128:(k + 1) * 128],
                                        in_=hout[:, k * 128:(k + 1) * 128],
                                        identity=ident[:])
                                ts = sp.tile([128, F1], F32, tag="ts",
                                             name=f"ts{layer}_{b}")
                                nc.scalar.copy(out=ts[:], in_=ps_tp[:])
                                ps_h2 = psH.tile([128, F2], F32, tag="h2",
                                                 space="PSUM", name=f"h2{layer}_{b}")
                                for k in range(KH):
                                    nc.tensor.matmul(
                                        out=ps_h2[:, 0:OUT],
                                        lhsT=ts[:, k * 128:(k + 1) * 128],
                                        rhs=w2sb[:, k, :],
                                        start=(k == 0), stop=(k == KH - 1))
                                for k in range(KH):
                                    nc.tensor.matmul(
                                        out=ps_h2[:, OUT:F2],
                                        lhsT=ts[:, (KH + k) * 128:(KH + k + 1) * 128],
                                        rhs=w2sb[:, k, :],
                                        start=(k == 0), stop=(k == KH - 1))
                                hw2sb = sp.tile([128, F2], GDT2, tag="hw2sb",
                                                name=f"hw2sb{layer}_{b}")
                                nc.scalar.copy(out=hw2sb[:], in_=ps_h2[:])
                                nc.sync.dma_start(
                                    out=hw2_sh[b * 128:(b + 1) * 128, :],
                                    in_=hw2sb[:])
                            else:
                                if b == 0:
                                    ps_cs = psC.tile([128, 1], F32, tag="cs",
                                                     space="PSUM")
                                    agg_layer.cs = ps_cs
                                else:
                                    ps_cs = agg_layer.cs
                                nc.tensor.matmul(
                                    out=ps_cs[:], lhsT=hout[:, 0:OUT],
                                    rhs=vdsb[:, b:b + 1],
                                    start=(b == 0), stop=(b == NB - 1),
                                    skip_group_check=True)
                                nc.sync.dma_start(
                                    out=h_sh[b * 128:(b + 1) * 128, :],
                                    in_=hout[:])
                        return post

                    if post_prev is not None:
                        post_prev()
                    post_prev = make_post()
                post_prev()

            agg_layer(1)
            allgather(hw2_sh, hw2_full, F2, GDT2)
            agg_layer(2)

            # ---------------- summary s and v = Wb @ s ----------------
            cssb = sp.tile([128, 1], F32, tag="cssb")
            nc.vector.tensor_copy(out=cssb[:], in_=agg_layer.cs[:])
            nc.sync.dma_start(out=cs_in[:, :], in_=cssb[:])
            if LOCAL_SIM:
                nc.sync.dma_start(out=cs_out[:, :], in_=cssb[:])
            else:
                nc.gpsimd.collective_compute(
                    "AllReduce", mybir.AluOpType.add,
                    replica_groups=[list(range(C))],
                    ins=[cs_in[:, :].opt()], outs=[cs_out[:, :].opt()])
            csr = sp.tile([128, 1], F32, tag="csr")
            nc.sync.dma_start(out=csr[:], in_=cs_out[:, :])
            ssb = sp.tile([128, 1], F32, tag="ssb")
            nc.scalar.activation(out=ssb[:], in_=csr[:],
                                 func=mybir.ActivationFunctionType.Sigmoid,
                                 scale=1.0 / N)
            ps_v = psC.tile([128, 1], F32, tag="cs", space="PSUM")
            nc.tensor.matmul(out=ps_v[:], lhsT=wbtsb[:], rhs=ssb[:],
                             start=True, stop=True)
            vsb = sp.tile([128, 1], F32, tag="vsb")
            nc.vector.tensor_copy(out=vsb[:], in_=ps_v[:])

            # ---------------- scores ----------------
            for b in range(NB):
                hb = sp.tile([128, F2], F32, tag="hb")
                (nc.scalar if SCL_RING else nc.sync).dma_start(
                    out=hb[:], in_=h_sh[b * 128:(b + 1) * 128, :])
                ps_tp = psT.tile([128, F1], F32, tag="tp", space="PSUM")
                for k in range(2):
                    nc.tensor.transpose(
                        out=ps_tp[:, k * 128:(k + 1) * 128],
                        in_=hb[:, k * OUT:k * OUT + 128],
                        identity=ident[:])
                ts2 = sp.tile([128, F2], F32, tag="ts2")
                nc.vector.tensor_copy(out=ts2[:], in_=ps_tp[:, :F2])
                ps_sc = psH.tile([128, F2], F32, tag="h2", space="PSUM")
                nc.tensor.matmul(out=ps_sc[:, 0:1], lhsT=ts2[:, 0:128],
                                 rhs=vsb[:], start=True, stop=True)
                nc.tensor.matmul(out=ps_sc[:, 1:2], lhsT=ts2[:, 128:256],
                                 rhs=vsb[:], start=True, stop=True)
                nc.vector.tensor_scalar(
                    out=sc_pos[:, b:b + 1], in0=ps_sc[:, 0:1],
                    scalar1=float(bb_val), scalar2=None, op0=mybir.AluOpType.add)
                nc.vector.tensor_scalar(
                    out=sc_neg[:, b:b + 1], in0=ps_sc[:, 1:2],
                    scalar1=float(bb_val), scalar2=None, op0=mybir.AluOpType.add)
            nc.sync.dma_start(out=out[0, :, :], in_=sc_pos[:])
            nc.sync.dma_start(out=out[1, :, :], in_=sc_neg[:])

    nc.compile()
    return nc


# ----------------------------------------------------------------------------
# entry point
# ----------------------------------------------------------------------------

_CACHE = {}


def _get_program(meta, HID, OUT, bias1_nz, bias2_nz, bb_val, C):
    key = (meta["N"], meta["E"], meta["IN"], HID, OUT, bias1_nz, bias2_nz,
           float(bb_val), C, meta["TOT"], meta["K"].tobytes())
    if key not in _CACHE:
        _CACHE[key] = _build(meta, HID, OUT, bias1_nz, bias2_nz, bb_val, C)
    return _CACHE[key]


def _make_in_maps(meta, arrs, W1, b1, W2, b2, Wb, C, bias1_nz, bias2_nz):
    in_maps = []
    for c in range(C):
        m = {
            "xtp": arrs["xT_pos"][c], "xtn": arrs["xT_neg"][c],
            "w1": W1, "w2": W2, "wbt": np.ascontiguousarray(Wb.T),
            "idx16": arrs["idx_dev"][c], "dstl": arrs["dst_dev"][c],
            "enorm": arrs["eno_dev"][c], "gcnt": arrs["gcnt_dev"][c][None, :],
            "snorm": arrs["sn_dev"][c], "valid": arrs["vd_dev"][c],
        }
        if bias1_nz:
            m["b1bc"] = np.tile(np.concatenate([b1, b1])[None, :], (128, 1))
        if bias2_nz:
            m["b2bc"] = np.tile(np.concatenate([b2, b2])[None, :], (128, 1))
        in_maps.append(m)
    return in_maps


def kernel(x, edge_index, perm, W1, b1, W2, b2, Wb, bb):
    C = 8
    x = np.asarray(x, np.float32)
    W1 = np.asarray(W1, np.float32)
    W2 = np.asarray(W2, np.float32)
    Wb = np.asarray(Wb, np.float32)
    b1 = np.asarray(b1, np.float32)
    b2 = np.asarray(b2, np.float32)
    bb_val = float(np.asarray(bb).reshape(-1)[0])
    N = x.shape[0]
    HID = W1.shape[1]
    OUT = W2.shape[1]

    meta, arrs = _prep(x, edge_index, perm, C)
    bias1_nz = bool(np.any(b1))
    bias2_nz = bool(np.any(b2))
    nc = _get_program(meta, HID, OUT, bias1_nz, bias2_nz, bb_val, C)
    in_maps = _make_in_maps(meta, arrs, W1, b1, W2, b2, Wb, C, bias1_nz, bias2_nz)

    res = bass_utils.run_bass_kernel_spmd(nc, in_maps, core_ids=list(range(C)))

    SH, NB = meta["SH"], meta["NB"]
    pos = np.empty((N, 1), np.float32)
    neg = np.empty((N, 1), np.float32)
    for c in range(C):
        sc = res.results[c]["scores"]
        pos[c * SH:(c + 1) * SH, 0] = sc[0].T.reshape(-1)[:SH]
        neg[c * SH:(c + 1) * SH, 0] = sc[1].T.reshape(-1)[:SH]
    return pos, neg

